# revision 4
# baseline (speedup 1.0000x reference)
"""ExtractOverlappingPatches Trainium2 kernel.

Input  x:   (16, 64, 128, 128) f32
Output y:   (16, 576, 128, 128) f32 where
            y[b, c*9 + (i*3+j), h, w] = x[b, c, h+i-1, w+j-1] (zero padded).

Strategy (pure memory movement, target_regime=memory):
  - Shard batch 16 -> 2 per core across 8 NeuronCores.
  - Per core: 2*64 = 128 input images of 128x128 -> one per SBUF partition,
    stored zero-padded to 130x130.  Output image index = p*9 + f where
    p = b*64 + c is exactly the input image index, so each of the 9 shifts
    is one strided SBUF -> DRAM DMA with long contiguous destination runs.
  - Loads ride the ACT HWDGE ring, stores the SP ring, striped over row
    chunks so the 8 MiB input read overlaps the 72 MiB output write.
  - Traffic per core: 8 MiB read + 72 MiB write ~= HBM roofline.
"""

import numpy as np

import concourse.bass as bass
import concourse.mybir as mybir
from concourse.bass_utils import run_bass_kernel_spmd

N_CORES = 8
B, C, H, W = 16, 64, 128, 128
PB = B // N_CORES  # batches per core
KH, KW = 3, 3
F = KH * KW
P = PB * C  # images per core == 128 partitions
HP, WP = H + 2, W + 2  # zero-padded image

STRIPE = 8  # rows per load chunk / store stripe

_cache = {}


def _build(stripe: int = STRIPE) -> bass.Bass:
    S = stripe
    L = H // S
    nc = bass.Bass()
    dt = mybir.dt.float32
    x = nc.dram_tensor("x", [PB, C, H, W], dt, kind="ExternalInput")
    out = nc.dram_tensor("out", [PB, C * F, H, W], dt, kind="ExternalOutput")

    x_im = x.rearrange("b c h w -> (b c) h w")
    # out channel index = c*F + f; merged (b c) stride is uniform because
    # stride_b = 576*img = 64 * (9*img) = 64 * stride_c.
    out_im = out.rearrange("b (c f) h w -> (b c) f h w", f=F)

    with (
        nc.sbuf_tensor([P, HP, WP], dt) as tile,
        nc.semaphore("vsem") as vsem,
        nc.semaphore("dsem") as dsem,
    ):
        lsems = [nc.alloc_semaphore(name=f"lsem{m}") for m in range(L)]
        with nc.Block() as block:

            @block.vector
            def _(vector):
                # Zero the 1-px border once; the shifted copies then carry
                # the zero padding out as part of dense contiguous writes.
                vector.memset(tile[:, 0, :], 0.0)
                vector.memset(tile[:, HP - 1, :], 0.0)
                vector.memset(tile[:, 1 : HP - 1, 0], 0.0)
                vector.memset(tile[:, 1 : HP - 1, WP - 1], 0.0).then_inc(vsem, 1)

            @block.scalar
            def _(scalar):
                # Load row chunks into the padded interior (ACT HWDGE ring).
                for m in range(L):
                    scalar.dma_start(
                        out=tile[:, m * S + 1 : (m + 1) * S + 1, 1 : W + 1],
                        in_=x_im[:, m * S : (m + 1) * S, :],
                    ).then_inc(lsems[m], 16)

            @block.sync
            def _(sync):
                sync.wait_ge(vsem, 1)
                waited = 0
                for k in range(L):
                    need = min(k + 2, L)  # stripe k reads chunks k-1..k+1
                    while waited < need:
                        sync.wait_ge(lsems[waited], 16)
                        waited += 1
                    for i in range(KH):
                        for j in range(KW):
                            f = i * KW + j
                            sync.dma_start(
                                out=out_im[:, f, k * S : (k + 1) * S, :],
                                in_=tile[:, k * S + i : (k + 1) * S + i, j : j + W],
                            ).then_inc(dsem, 16)
                sync.wait_ge(dsem, L * F * 16)

        for s in lsems:
            nc.release_semaphore(s)

    return nc


def kernel(x) -> np.ndarray:
    x = np.asarray(x, dtype=np.float32)
    assert x.shape == (B, C, H, W)
    if "nc" not in _cache:
        _cache["nc"] = _build()
    nc = _cache["nc"]
    in_maps = [
        {"x": np.ascontiguousarray(x[i * PB : (i + 1) * PB])} for i in range(N_CORES)
    ]
    res = run_bass_kernel_spmd(nc, in_maps, list(range(N_CORES)))
    return np.concatenate([r["out"] for r in res.results], axis=0)


# revision 6
# speedup vs baseline: 2.5672x; 2.5672x over previous
"""ExtractOverlappingPatches Trainium2 kernel.

Input  x:   (16, 64, 128, 128) f32
Output y:   (16, 576, 128, 128) f32 where
            y[b, c*9 + (i*3+j), h, w] = x[b, c, h+i-1, w+j-1] (zero padded).

Strategy (pure memory movement, target_regime=memory):
  - Shard batch 16 -> 2 per core across 8 NeuronCores.
  - Per core: 2*64 = 128 input images of 128x128 -> one per SBUF partition,
    stored zero-padded to 130x130.  Output image index = p*9 + f where
    p = b*64 + c is exactly the input image index, so each of the 9 shifts
    is a regular strided SBUF -> DRAM DMA with contiguous destination runs.
  - Input load is striped over row chunks and overlapped with stores.
  - Stores are spread over all three DMA issuers (SP HWDGE, ACT HWDGE,
    gpsimd SWDGE) so descriptor generation and queue draining parallelize.
  - Traffic per core: 8 MiB read + 72 MiB write (the irreducible minimum).
"""

import numpy as np

import concourse.bass as bass
import concourse.mybir as mybir
from concourse.bass_utils import run_bass_kernel_spmd

N_CORES = 8
B, C, H, W = 16, 64, 128, 128
PB = B // N_CORES  # batches per core
KH, KW = 3, 3
F = KH * KW
P = PB * C  # images per core == 128 partitions
HP, WP = H + 2, W + 2  # zero-padded image

STRIPE = 8  # rows per load chunk / store stripe

_cache = {}


def _build(stripe: int = STRIPE) -> bass.Bass:
    S = stripe
    L = H // S
    nc = bass.Bass()
    dt = mybir.dt.float32
    x = nc.dram_tensor("x", [PB, C, H, W], dt, kind="ExternalInput")
    out = nc.dram_tensor("out", [PB, C * F, H, W], dt, kind="ExternalOutput")

    x_im = x.rearrange("b c h w -> (b c) h w")
    # out channel index = c*F + f; merged (b c) stride is uniform because
    # stride_b = 576*img = 64 * (9*img) = 64 * stride_c.
    out_im = out.rearrange("b (c f) h w -> (b c) f h w", f=F)

    # Store work list: stripe k / shift (i, j) needs load chunks 0..k+1.
    work = [
        (k, i, j, min(k + 2, L))
        for k in range(L)
        for i in range(KH)
        for j in range(KW)
    ]
    # ACT also carries the 8 MiB of loads, so it gets a smaller store share
    # such that all three issuers finish together (ACT: loads + ~18.7 MiB,
    # SP/gpsimd: ~26.7 MiB each).
    n_act = (len(work) * 37) // 144
    idx = set(np.linspace(0, len(work) - 1, n_act).round().astype(int).tolist())
    act_share = [w for q, w in enumerate(work) if q in idx]
    rest = [w for q, w in enumerate(work) if q not in idx]
    shares = [rest[0::2], act_share, rest[1::2]]  # SP / ACT / gpsimd

    with (
        nc.sbuf_tensor([P, HP, WP], dt) as tile,
        nc.semaphore("vsem") as vsem,
        nc.semaphore("dsem") as dsem,
        nc.semaphore("gsem") as gsem,
    ):
        lsems = [nc.alloc_semaphore(name=f"lsem{m}") for m in range(L)]
        with nc.Block() as block:

            @block.vector
            def _(vector):
                # Zero the 1-px border once; the shifted copies then carry
                # the zero padding out as part of dense contiguous writes.
                vector.memset(tile[:, 0, :], 0.0)
                vector.memset(tile[:, HP - 1, :], 0.0)
                vector.memset(tile[:, 1 : HP - 1, 0], 0.0)
                vector.memset(tile[:, 1 : HP - 1, WP - 1], 0.0).then_inc(vsem, 1)

            def emit_stores(eng, lst, sem):
                waited = 0
                eng.wait_ge(vsem, 1)
                for k, i, j, need in lst:
                    while waited < need:
                        eng.wait_ge(lsems[waited], 16)
                        waited += 1
                    f = i * KW + j
                    eng.dma_start(
                        out=out_im[:, f, k * S : (k + 1) * S, :],
                        in_=tile[:, k * S + i : (k + 1) * S + i, j : j + W],
                    ).then_inc(sem, 16)

            @block.scalar
            def _(scalar):
                # Load row chunks into the padded interior, then this ring's
                # share of the stores.
                for m in range(L):
                    scalar.dma_start(
                        out=tile[:, m * S + 1 : (m + 1) * S + 1, 1 : W + 1],
                        in_=x_im[:, m * S : (m + 1) * S, :],
                    ).then_inc(lsems[m], 16)
                emit_stores(scalar, shares[1], dsem)

            @block.gpsimd
            def _(gpsimd):
                emit_stores(gpsimd, shares[2], gsem)

            @block.sync
            def _(sync):
                emit_stores(sync, shares[0], dsem)
                sync.wait_ge(dsem, (len(shares[0]) + len(shares[1])) * 16)
                sync.wait_ge(gsem, len(shares[2]) * 16)

        for s in lsems:
            nc.release_semaphore(s)

    return nc


def kernel(x) -> np.ndarray:
    x = np.asarray(x, dtype=np.float32)
    assert x.shape == (B, C, H, W)
    if "nc" not in _cache:
        _cache["nc"] = _build()
    nc = _cache["nc"]
    in_maps = [
        {"x": np.ascontiguousarray(x[i * PB : (i + 1) * PB])} for i in range(N_CORES)
    ]
    res = run_bass_kernel_spmd(nc, in_maps, list(range(N_CORES)))
    return np.concatenate([r["out"] for r in res.results], axis=0)


# revision 7
# speedup vs baseline: 2.6127x; 1.0177x over previous
"""ExtractOverlappingPatches Trainium2 kernel.

Input  x:   (16, 64, 128, 128) f32
Output y:   (16, 576, 128, 128) f32 where
            y[b, c*9 + (i*3+j), h, w] = x[b, c, h+i-1, w+j-1] (zero padded).

Strategy (pure memory movement, target_regime=memory):
  - Shard batch 16 -> 2 per core across 8 NeuronCores.
  - Per core: 2*64 = 128 input images of 128x128 -> one per SBUF partition,
    stored zero-padded to 130x130.  Output image index = p*9 + f where
    p = b*64 + c is exactly the input image index, so each of the 9 shifts
    is a regular strided SBUF -> DRAM DMA with contiguous destination runs.
  - Input load is striped over row chunks and overlapped with stores.
  - Stores are spread over all three DMA issuers (SP HWDGE, ACT HWDGE,
    gpsimd SWDGE) so descriptor generation and queue draining parallelize.
  - Traffic per core: 8 MiB read + 72 MiB write (the irreducible minimum).
"""

import numpy as np

import concourse.bass as bass
import concourse.mybir as mybir
from concourse.bass_utils import run_bass_kernel_spmd

N_CORES = 8
B, C, H, W = 16, 64, 128, 128
PB = B // N_CORES  # batches per core
KH, KW = 3, 3
F = KH * KW
P = PB * C  # images per core == 128 partitions
HP, WP = H + 2, W + 2  # zero-padded image

STRIPE = 8  # rows per load chunk / store stripe

_cache = {}


def _build(stripe: int = STRIPE) -> bass.Bass:
    S = stripe
    L = H // S
    nc = bass.Bass()
    dt = mybir.dt.float32
    x = nc.dram_tensor("x", [PB, C, H, W], dt, kind="ExternalInput")
    out = nc.dram_tensor("out", [PB, C * F, H, W], dt, kind="ExternalOutput")

    x_im = x.rearrange("b c h w -> (b c) h w")
    # out channel index = c*F + f; merged (b c) stride is uniform because
    # stride_b = 576*img = 64 * (9*img) = 64 * stride_c.
    out_im = out.rearrange("b (c f) h w -> (b c) f h w", f=F)

    # Store work list: stripe k / shift (i, j) needs load chunks 0..k+1.
    work = [
        (k, i, j, min(k + 2, L))
        for k in range(L)
        for i in range(KH)
        for j in range(KW)
    ]
    # ACT also carries the 8 MiB of loads, so it gets a smaller store share
    # such that all three issuers finish together (ACT: loads + ~19.5 MiB,
    # SP/gpsimd: ~26 MiB each; 39/144 tuned via CoreSim sweep).
    n_act = (len(work) * 39) // 144
    idx = set(np.linspace(0, len(work) - 1, n_act).round().astype(int).tolist())
    act_share = [w for q, w in enumerate(work) if q in idx]
    rest = [w for q, w in enumerate(work) if q not in idx]
    shares = [rest[0::2], act_share, rest[1::2]]  # SP / ACT / gpsimd

    with (
        nc.sbuf_tensor([P, HP, WP], dt) as tile,
        nc.semaphore("vsem") as vsem,
        nc.semaphore("dsem") as dsem,
        nc.semaphore("gsem") as gsem,
    ):
        lsems = [nc.alloc_semaphore(name=f"lsem{m}") for m in range(L)]
        with nc.Block() as block:

            @block.vector
            def _(vector):
                # Zero the 1-px border once; the shifted copies then carry
                # the zero padding out as part of dense contiguous writes.
                vector.memset(tile[:, 0, :], 0.0)
                vector.memset(tile[:, HP - 1, :], 0.0)
                vector.memset(tile[:, 1 : HP - 1, 0], 0.0)
                vector.memset(tile[:, 1 : HP - 1, WP - 1], 0.0).then_inc(vsem, 1)

            def emit_stores(eng, lst, sem):
                waited = 0
                eng.wait_ge(vsem, 1)
                for k, i, j, need in lst:
                    while waited < need:
                        eng.wait_ge(lsems[waited], 16)
                        waited += 1
                    f = i * KW + j
                    eng.dma_start(
                        out=out_im[:, f, k * S : (k + 1) * S, :],
                        in_=tile[:, k * S + i : (k + 1) * S + i, j : j + W],
                    ).then_inc(sem, 16)

            @block.scalar
            def _(scalar):
                # Load row chunks into the padded interior, then this ring's
                # share of the stores.
                for m in range(L):
                    scalar.dma_start(
                        out=tile[:, m * S + 1 : (m + 1) * S + 1, 1 : W + 1],
                        in_=x_im[:, m * S : (m + 1) * S, :],
                    ).then_inc(lsems[m], 16)
                emit_stores(scalar, shares[1], dsem)

            @block.gpsimd
            def _(gpsimd):
                emit_stores(gpsimd, shares[2], gsem)

            @block.sync
            def _(sync):
                emit_stores(sync, shares[0], dsem)
                sync.wait_ge(dsem, (len(shares[0]) + len(shares[1])) * 16)
                sync.wait_ge(gsem, len(shares[2]) * 16)

        for s in lsems:
            nc.release_semaphore(s)

    return nc


def kernel(x) -> np.ndarray:
    x = np.asarray(x, dtype=np.float32)
    assert x.shape == (B, C, H, W)
    if "nc" not in _cache:
        _cache["nc"] = _build()
    nc = _cache["nc"]
    in_maps = [
        {"x": np.ascontiguousarray(x[i * PB : (i + 1) * PB])} for i in range(N_CORES)
    ]
    res = run_bass_kernel_spmd(nc, in_maps, list(range(N_CORES)))
    return np.concatenate([r["out"] for r in res.results], axis=0)


# revision 9
# speedup vs baseline: 2.6640x; 1.0196x over previous
"""ExtractOverlappingPatches Trainium2 kernel.

Input  x:   (16, 64, 128, 128) f32
Output y:   (16, 576, 128, 128) f32 where
            y[b, c*9 + (i*3+j), h, w] = x[b, c, h+i-1, w+j-1] (zero padded).

Strategy (pure memory movement, target_regime=memory):
  - Shard batch 16 -> 2 per core across 8 NeuronCores.
  - Per core: 2*64 = 128 input images of 128x128 -> one per SBUF partition,
    stored zero-padded to 130x130.  Output image index = p*9 + f where
    p = b*64 + c is exactly the input image index, so each of the 9 shifts
    is a regular strided SBUF -> DRAM DMA with contiguous destination runs.
  - Input load is striped over row chunks and overlapped with stores.
  - Stores are spread over all three DMA issuers (SP HWDGE, ACT HWDGE,
    gpsimd SWDGE) so descriptor generation and queue draining parallelize.
  - Traffic per core: 8 MiB read + 72 MiB write (the irreducible minimum).
"""

import numpy as np

import concourse.bass as bass
import concourse.mybir as mybir
from concourse.bass_utils import run_bass_kernel_spmd

N_CORES = 8
B, C, H, W = 16, 64, 128, 128
PB = B // N_CORES  # batches per core
KH, KW = 3, 3
F = KH * KW
P = PB * C  # images per core == 128 partitions
HP, WP = H + 2, W + 2  # zero-padded image

STRIPE = 8  # rows per load chunk / store stripe

_cache = {}


def _build(stripe: int = STRIPE) -> bass.Bass:
    S = stripe
    L = H // S
    nc = bass.Bass()
    dt = mybir.dt.float32
    x = nc.dram_tensor("x", [PB, C, H, W], dt, kind="ExternalInput")
    out = nc.dram_tensor("out", [PB, C * F, H, W], dt, kind="ExternalOutput")

    x_im = x.rearrange("b c h w -> (b c) h w")
    # out channel index = c*F + f; merged (b c) stride is uniform because
    # stride_b = 576*img = 64 * (9*img) = 64 * stride_c.
    out_im = out.rearrange("b (c f) h w -> (b c) f h w", f=F)

    # Store work list: stripe k / shift (i, j) needs load chunks 0..k+1.
    work = [
        (k, i, j, min(k + 2, L))
        for k in range(L)
        for i in range(KH)
        for j in range(KW)
    ]
    # Loads and stores are both dealt round-robin across the three issuers,
    # so each ring carries (8 + 72)/3 MiB and they all finish together.
    shares = [work[r::3] for r in range(3)]  # SP / ACT / gpsimd
    load_shares = [list(range(L))[r::3] for r in range(3)]

    with (
        nc.sbuf_tensor([P, HP, WP], dt) as tile,
        nc.semaphore("vsem") as vsem,
        nc.semaphore("dsem") as dsem,
        nc.semaphore("gsem") as gsem,
    ):
        lsems = [nc.alloc_semaphore(name=f"lsem{m}") for m in range(L)]
        with nc.Block() as block:

            @block.vector
            def _(vector):
                # Zero the 1-px border once; the shifted copies then carry
                # the zero padding out as part of dense contiguous writes.
                vector.memset(tile[:, 0, :], 0.0)
                vector.memset(tile[:, HP - 1, :], 0.0)
                vector.memset(tile[:, 1 : HP - 1, 0], 0.0)
                vector.memset(tile[:, 1 : HP - 1, WP - 1], 0.0).then_inc(vsem, 1)

            def emit_loads(eng, ms):
                # Load this ring's row chunks into the padded interior.
                for m in ms:
                    eng.dma_start(
                        out=tile[:, m * S + 1 : (m + 1) * S + 1, 1 : W + 1],
                        in_=x_im[:, m * S : (m + 1) * S, :],
                    ).then_inc(lsems[m], 16)

            def emit_stores(eng, lst, sem):
                waited = 0
                eng.wait_ge(vsem, 1)
                for k, i, j, need in lst:
                    while waited < need:
                        eng.wait_ge(lsems[waited], 16)
                        waited += 1
                    f = i * KW + j
                    eng.dma_start(
                        out=out_im[:, f, k * S : (k + 1) * S, :],
                        in_=tile[:, k * S + i : (k + 1) * S + i, j : j + W],
                    ).then_inc(sem, 16)

            @block.scalar
            def _(scalar):
                emit_loads(scalar, load_shares[1])
                emit_stores(scalar, shares[1], dsem)

            @block.gpsimd
            def _(gpsimd):
                emit_loads(gpsimd, load_shares[2])
                emit_stores(gpsimd, shares[2], gsem)

            @block.sync
            def _(sync):
                emit_loads(sync, load_shares[0])
                emit_stores(sync, shares[0], dsem)
                sync.wait_ge(dsem, (len(shares[0]) + len(shares[1])) * 16)
                sync.wait_ge(gsem, len(shares[2]) * 16)

        for s in lsems:
            nc.release_semaphore(s)

    return nc


def kernel(x) -> np.ndarray:
    x = np.asarray(x, dtype=np.float32)
    assert x.shape == (B, C, H, W)
    if "nc" not in _cache:
        _cache["nc"] = _build()
    nc = _cache["nc"]
    in_maps = [
        {"x": np.ascontiguousarray(x[i * PB : (i + 1) * PB])} for i in range(N_CORES)
    ]
    res = run_bass_kernel_spmd(nc, in_maps, list(range(N_CORES)))
    return np.concatenate([r["out"] for r in res.results], axis=0)


# revision 10
# speedup vs baseline: 2.6866x; 1.0085x over previous
"""ExtractOverlappingPatches Trainium2 kernel.

Input  x:   (16, 64, 128, 128) f32
Output y:   (16, 576, 128, 128) f32 where
            y[b, c*9 + (i*3+j), h, w] = x[b, c, h+i-1, w+j-1] (zero padded).

Strategy (pure memory movement, target_regime=memory):
  - Shard batch 16 -> 2 per core across 8 NeuronCores.
  - Per core: 2*64 = 128 input images of 128x128 -> one per SBUF partition,
    stored zero-padded to 130x130.  Output image index = p*9 + f where
    p = b*64 + c is exactly the input image index, so each of the 9 shifts
    is a regular strided SBUF -> DRAM DMA with contiguous destination runs.
  - Input load is striped over row chunks and overlapped with stores.
  - Stores are spread over all three DMA issuers (SP HWDGE, ACT HWDGE,
    gpsimd SWDGE) so descriptor generation and queue draining parallelize.
  - Traffic per core: 8 MiB read + 72 MiB write (the irreducible minimum).
"""

import numpy as np

import concourse.bass as bass
import concourse.mybir as mybir
from concourse.bass_utils import run_bass_kernel_spmd

N_CORES = 8
B, C, H, W = 16, 64, 128, 128
PB = B // N_CORES  # batches per core
KH, KW = 3, 3
F = KH * KW
P = PB * C  # images per core == 128 partitions
HP, WP = H + 2, W + 2  # zero-padded image

STRIPE = 4  # rows per load chunk / store stripe

_cache = {}


def _build(stripe: int = STRIPE) -> bass.Bass:
    S = stripe
    L = H // S
    nc = bass.Bass()
    dt = mybir.dt.float32
    x = nc.dram_tensor("x", [PB, C, H, W], dt, kind="ExternalInput")
    out = nc.dram_tensor("out", [PB, C * F, H, W], dt, kind="ExternalOutput")

    x_im = x.rearrange("b c h w -> (b c) h w")
    # out channel index = c*F + f; merged (b c) stride is uniform because
    # stride_b = 576*img = 64 * (9*img) = 64 * stride_c.
    out_im = out.rearrange("b (c f) h w -> (b c) f h w", f=F)

    # Store work list: stripe k / shift (i, j) needs load chunks 0..k+1.
    work = [
        (k, i, j, min(k + 2, L))
        for k in range(L)
        for i in range(KH)
        for j in range(KW)
    ]
    # Loads and stores are both dealt round-robin across the three issuers,
    # so each ring carries (8 + 72)/3 MiB and they all finish together.
    shares = [work[r::3] for r in range(3)]  # SP / ACT / gpsimd
    load_shares = [list(range(L))[r::3] for r in range(3)]

    with (
        nc.sbuf_tensor([P, HP, WP], dt) as tile,
        nc.semaphore("vsem") as vsem,
        nc.semaphore("dsem") as dsem,
        nc.semaphore("gsem") as gsem,
    ):
        lsems = [nc.alloc_semaphore(name=f"lsem{m}") for m in range(L)]
        with nc.Block() as block:

            @block.vector
            def _(vector):
                # Zero the 1-px border once; the shifted copies then carry
                # the zero padding out as part of dense contiguous writes.
                vector.memset(tile[:, 0, :], 0.0)
                vector.memset(tile[:, HP - 1, :], 0.0)
                vector.memset(tile[:, 1 : HP - 1, 0], 0.0)
                vector.memset(tile[:, 1 : HP - 1, WP - 1], 0.0).then_inc(vsem, 1)

            def emit_loads(eng, ms):
                # Load this ring's row chunks into the padded interior.
                for m in ms:
                    eng.dma_start(
                        out=tile[:, m * S + 1 : (m + 1) * S + 1, 1 : W + 1],
                        in_=x_im[:, m * S : (m + 1) * S, :],
                    ).then_inc(lsems[m], 16)

            def emit_stores(eng, lst, sem):
                waited = 0
                eng.wait_ge(vsem, 1)
                for k, i, j, need in lst:
                    while waited < need:
                        eng.wait_ge(lsems[waited], 16)
                        waited += 1
                    f = i * KW + j
                    eng.dma_start(
                        out=out_im[:, f, k * S : (k + 1) * S, :],
                        in_=tile[:, k * S + i : (k + 1) * S + i, j : j + W],
                    ).then_inc(sem, 16)

            @block.scalar
            def _(scalar):
                emit_loads(scalar, load_shares[1])
                emit_stores(scalar, shares[1], dsem)

            @block.gpsimd
            def _(gpsimd):
                emit_loads(gpsimd, load_shares[2])
                emit_stores(gpsimd, shares[2], gsem)

            @block.sync
            def _(sync):
                emit_loads(sync, load_shares[0])
                emit_stores(sync, shares[0], dsem)
                sync.wait_ge(dsem, (len(shares[0]) + len(shares[1])) * 16)
                sync.wait_ge(gsem, len(shares[2]) * 16)

        for s in lsems:
            nc.release_semaphore(s)

    return nc


def kernel(x) -> np.ndarray:
    x = np.asarray(x, dtype=np.float32)
    assert x.shape == (B, C, H, W)
    if "nc" not in _cache:
        _cache["nc"] = _build()
    nc = _cache["nc"]
    in_maps = [
        {"x": np.ascontiguousarray(x[i * PB : (i + 1) * PB])} for i in range(N_CORES)
    ]
    res = run_bass_kernel_spmd(nc, in_maps, list(range(N_CORES)))
    return np.concatenate([r["out"] for r in res.results], axis=0)
